# revision 1
# baseline (speedup 1.0000x reference)
"""Trainium2 Bass kernel for nn_LongTermAccompanimentBeatwiseUpcomingBars.

6-layer transformer decoder (D=512, 8 heads, FFN=2048) over B=16, T=512 with
S=256 cross-attention memory, causal + banded masks, three 512->9 heads.

Sharding: data-parallel over batch, 2 batches per core across 8 cores.

Device-side design:
  - all matmuls in float32r (full PE rate at moving-dim >= 256, ~1.5e-4 rel)
  - residual stream token-major in SBUF; feature-major copies via PE transpose
  - attention with TRANSPOSED scores S_T[tk, tq]: softmax sums come free from
    a ones column appended to V; no attn transpose needed; normalization via
    broadcast-DMA of reciprocal sums and in-place multiply
  - cross-attention K/V truncated to 32 memory positions (banded mask never
    allows s >= 31)
  - masks added into PSUM scores by DVE; exp on ACT (single LUT table);
    layernorm via bn_stats/bn_aggr + fast-rsqrt Newton on DVE
  - 1/sqrt(hd) folded into q-side weights on the host
"""

import numpy as np
from contextlib import ExitStack

import concourse.bass as bass
import concourse.mybir as mybir
import concourse.tile as tile
from concourse import bacc
from concourse.bass_utils import run_bass_kernel_spmd

F32 = mybir.dt.float32
F32R = mybir.dt.float32r
I32 = mybir.dt.int32
AF = mybir.ActivationFunctionType
OP = mybir.AluOpType

NCORES = 8
B = 16
BL = B // NCORES   # 2
T = 512
D = 512
NH = 8
HD = D // NH       # 64
FF = 2048
V = 9
VO = 3 * V         # 27
VOP = 32           # heads output padded for f32r ISA alignment
L = 6
P = 128
SM = 32            # truncated cross-attn memory
SMB = BL * SM      # 64
TQ = BL * T        # 1024
NT = TQ // P       # 8
NTB = T // P       # 4
NF = D // P        # 4
NFF = FF // P      # 16
FH = FF // 2       # 1024 per ffn half
NFH = FH // P      # 8
EPS = 1e-5
NEG = -1.0e30

_PROG_CACHE: dict = {}


def _build_program(flags, n_layers=L, debug_dump=False):
    has_bias, has_ln_scale, has_ln_bias = flags
    nc = bacc.Bacc("TRN2", target_bir_lowering=False, debug=False)

    d_tgtT = nc.dram_tensor("tgtT", [VO, TQ], F32R, kind="ExternalInput")
    d_pe = nc.dram_tensor("pe_eb", [T, D], F32, kind="ExternalInput")
    d_embT = nc.dram_tensor("emb_wT", [VO, D], F32R, kind="ExternalInput")
    d_memT = nc.dram_tensor("memT", [D, SMB], F32R, kind="ExternalInput")
    d_msa = nc.dram_tensor("mask_sa", [P, P], F32, kind="ExternalInput")
    d_mca = nc.dram_tensor("mask_ca", [SM, T], F32, kind="ExternalInput")
    d_sel = nc.dram_tensor("head_sel", [NH, NF * P], F32R, kind="ExternalInput")
    d_ident = nc.dram_tensor("identity", [P, P], F32R, kind="ExternalInput")
    d_ones = nc.dram_tensor("const_ones", [P, P], F32R, kind="ExternalInput")

    def wt(name, shape):
        return nc.dram_tensor(name, shape, F32, kind="ExternalInput")

    def wtr(name, shape):
        # f32r-typed DRAM input: same bits as f32, lets HWDGE load without a
        # cast while satisfying the f32r-producer rule for matmul inputs
        return nc.dram_tensor(name, shape, F32R, kind="ExternalInput")

    d_wq = wtr("wq", [L, D, D])
    d_wk = wtr("wk", [L, D, D])
    d_wv = wtr("wv", [L, D, D])
    d_wo = wtr("wo", [L, D, D])
    d_cq = wtr("cq", [L, D, D])
    d_ck = wtr("ck", [L, D, D])
    d_cv = wtr("cv", [L, D, D])
    d_co = wtr("co", [L, D, D])
    d_w1 = wtr("w1", [L, D, FF])
    d_w2 = wtr("w2", [L, FF, D])
    d_hw = wtr("hwT", [D, VOP])
    if has_bias:
        d_bq = wt("bq", [L, 1, D])
        d_bk = wt("bk", [L, 1, D])
        d_bv = wtr("bv", [L, 1, D])
        d_bo = wtr("bo", [L, 1, D])
        d_cbq = wt("cbq", [L, 1, D])
        d_cbk = wt("cbk", [L, 1, D])
        d_cbv = wtr("cbv", [L, 1, D])
        d_cbo = wtr("cbo", [L, 1, D])
        d_b1 = wt("b1", [L, 1, FF])
        d_b2 = wtr("b2", [L, 1, D])
        d_hb = wtr("hb", [1, VOP])
    if has_ln_scale:
        d_ls = wt("ln_s", [3 * L + 1, 1, D])
    if has_ln_bias:
        d_lb = wt("ln_b", [3 * L + 1, 1, D])

    d_out = nc.dram_tensor("out", [TQ, VO], F32, kind="ExternalOutput")
    if debug_dump:
        d_dbg = nc.dram_tensor("dbg_x", [TQ, D], F32, kind="ExternalOutput")

    with tile.TileContext(nc) as tc, ExitStack() as ctx:
        pool = ctx.enter_context(tc.tile_pool(name="persist", bufs=1))
        wpool = ctx.enter_context(tc.tile_pool(name="wres", bufs=1))
        wstr = ctx.enter_context(tc.tile_pool(name="wstr", bufs=3))
        xtp = ctx.enter_context(tc.tile_pool(name="xT", bufs=1))
        qkp = ctx.enter_context(tc.tile_pool(name="qk", bufs=1))
        otp = ctx.enter_context(tc.tile_pool(name="oT", bufs=1))
        expp = ctx.enter_context(tc.tile_pool(name="expT", bufs=3))
        lnp = ctx.enter_context(tc.tile_pool(name="lnt", bufs=3))
        smallp = ctx.enter_context(tc.tile_pool(name="small", bufs=2))
        psp = ctx.enter_context(tc.tile_pool(name="ps", bufs=7, space="PSUM"))

        # ---- constants --------------------------------------------------
        tgtT = pool.tile([VO, TQ], F32R)
        nc.sync.dma_start(tgtT[:], d_tgtT.ap())
        embT = pool.tile([VO, D], F32R)
        nc.sync.dma_start(embT[:], d_embT.ap())
        memT = pool.tile([P, NF, SMB], F32R)
        nc.sync.dma_start(memT[:], d_memT.ap().rearrange("(c p) s -> p c s", p=P))
        msa = pool.tile([P, P], F32)
        nc.sync.dma_start(msa[:], d_msa.ap())
        mca = pool.tile([SM, T], F32)
        nc.sync.dma_start(mca[:], d_mca.ap())
        ident = pool.tile([P, P], F32R)
        nc.sync.dma_start(ident[:], d_ident.ap())
        ones_pp = pool.tile([P, P], F32R)
        nc.sync.dma_start(ones_pp[:], d_ones.ap())
        ones_r = ones_pp[0:1, :]
        sel_sb = pool.tile([NH, NF, P], F32R)
        nc.sync.dma_start(
            sel_sb[:], d_sel.ap().rearrange("h (c p) -> h c p", p=P))
        hw_sb = pool.tile([P, NF, VOP], F32R)
        nc.sync.dma_start(hw_sb[:], d_hw.ap().rearrange("(c p) o -> p c o", p=P))
        if has_bias:
            hb_sb = pool.tile([1, VOP], F32R)
            nc.sync.dma_start(hb_sb[:], d_hb.ap())

        x_sb = pool.tile([P, NT, D], F32R)   # residual stream, token-major
        r_sb = pool.tile([P, NT, D], F32)    # pre-norm sum
        st6 = pool.tile([P, NT, 6], F32)
        mv2 = pool.tile([P, NT, 2], F32)
        rstd = pool.tile([P, NT], F32)
        nmean = pool.tile([P, NT], F32)
        scr_i = pool.tile([P, NT], I32)
        scr_f = pool.tile([P, NT], F32)
        scr_g = pool.tile([P, NT], F32)

        # ---- helpers ----------------------------------------------------

        def stream_w_oc(dram, l, oc, tag):
            """[P, NF, P] chunk of a [D, D] W.T: output cols oc*P..+P"""
            t = wstr.tile([P, NF, P], F32R, tag=tag)
            nc.sync.dma_start(
                t[:],
                dram.ap()[l].rearrange("(c p) o -> p c o", p=P)[:, :, oc * P:(oc + 1) * P])
            return t

        def load_w_full(dram, l, n_in, n_out, tag):
            t = wpool.tile([P, n_in // P, n_out], F32R, tag=tag)
            nc.sync.dma_start(
                t[:], dram.ap()[l].rearrange("(c p) o -> p c o", p=P))
            return t

        def load_b_row(dram, l, n, tag):
            t = wpool.tile([1, n], F32R, tag=tag)
            nc.sync.dma_start(t[:], dram.ap()[l])
            return t

        def bias_cols(dram, l, n, tag):
            t = wpool.tile([P, n // P], F32, tag=tag)
            nc.sync.dma_start(
                t[:], dram.ap()[l, 0].rearrange("(c p) -> p c", p=P))
            return t

        def transpose_x(src):
            """token-major [P, NT, D] f32r -> feature-major [P, NF, TQ] f32r.
            4 transposed blocks land in one PSUM bank, evacuated by a single
            strided ACT copy."""
            dst = xtp.tile([P, NF, TQ], F32R, tag="xT")
            for tcn in range(NT):
                tp = psp.tile([P, NF * P], F32R, tag="ps")
                for fc in range(NF):
                    nc.tensor.transpose(
                        tp[:, fc * P:(fc + 1) * P],
                        src[:, tcn, fc * P:(fc + 1) * P], ident[:])
                nc.scalar.copy(
                    dst[:, :, tcn * P:(tcn + 1) * P],
                    tp[:].rearrange("p (f c) -> p f c", f=NF))
            return dst

        def proj_fmajor_b(xT, w_dram, l, b, out_t, bias_sc, wtag):
            """feature-major projection for batch b's token columns."""
            for oc in range(NF):
                w_oc = stream_w_oc(w_dram, l, oc, wtag)
                pt = psp.tile([P, T], F32, tag="ps")
                for ic in range(NF):
                    nc.tensor.matmul(
                        pt[:], w_oc[:, ic, :], xT[:, ic, b * T:(b + 1) * T],
                        start=(ic == 0), stop=(ic == NF - 1))
                if bias_sc is not None:
                    nc.scalar.activation(
                        out_t[:, oc, :], pt[:], AF.Identity,
                        bias=bias_sc[:, oc:oc + 1], scale=1.0)
                else:
                    nc.scalar.copy(out_t[:, oc, :], pt[:])

        def ln_block(ln_idx, out_x):
            for tcn in range(NT):
                nc.vector.bn_stats(st6[:, tcn], r_sb[:, tcn])
            for tcn in range(NT):
                nc.vector.bn_aggr(mv2[:, tcn], st6[:, tcn])
            ve = scr_f
            nc.vector.tensor_scalar_add(ve[:], mv2[:, :, 1], float(EPS))
            nc.vector.tensor_scalar(
                scr_i[:], ve[:].bitcast(I32), 1, None, OP.logical_shift_right)
            nc.vector.tensor_scalar(
                scr_i[:], scr_i[:], -1, 0x5F3759DF, OP.mult, OP.add)
            y = rstd
            nc.vector.tensor_copy(y[:], scr_i[:].bitcast(F32))
            for _ in range(2):
                nc.vector.tensor_tensor(scr_g[:], ve[:], y[:], OP.mult)
                nc.vector.tensor_tensor(scr_g[:], scr_g[:], y[:], OP.mult)
                nc.vector.tensor_scalar(
                    scr_g[:], scr_g[:], -0.5, 1.5, OP.mult, OP.add)
                nc.vector.tensor_tensor(y[:], y[:], scr_g[:], OP.mult)
            nc.vector.tensor_scalar_mul(nmean[:], mv2[:, :, 0], -1.0)

            s_rep = b_rep = None
            if has_ln_scale:
                s_rep = wpool.tile([P, D], F32, tag="ln_s_rep")
                nc.sync.dma_start(
                    s_rep[:], d_ls.ap()[ln_idx, 0].to_broadcast((P, D)))
            if has_ln_bias:
                b_rep = wpool.tile([P, D], F32, tag="ln_b_rep")
                nc.sync.dma_start(
                    b_rep[:], d_lb.ap()[ln_idx, 0].to_broadcast((P, D)))

            for tcn in range(NT):
                t1 = lnp.tile([P, D], F32, tag="t1")
                nc.scalar.activation(
                    t1[:], r_sb[:, tcn], AF.Identity,
                    bias=nmean[:, tcn:tcn + 1], scale=1.0)
                if has_ln_scale:
                    tgt_ap = out_x[:, tcn]
                    if has_ln_bias:
                        t2 = lnp.tile([P, D], F32, tag="t2")
                        nc.vector.scalar_tensor_tensor(
                            t2[:], t1[:], rstd[:, tcn:tcn + 1], s_rep[:],
                            OP.mult, OP.mult)
                        nc.gpsimd.tensor_tensor(
                            tgt_ap, t2[:], b_rep[:], OP.add)
                    else:
                        nc.vector.scalar_tensor_tensor(
                            tgt_ap, t1[:], rstd[:, tcn:tcn + 1], s_rep[:],
                            OP.mult, OP.mult)
                else:
                    if has_ln_bias:
                        t2 = lnp.tile([P, D], F32, tag="t2")
                        nc.vector.tensor_scalar_mul(
                            t2[:], t1[:], rstd[:, tcn:tcn + 1])
                        nc.gpsimd.tensor_tensor(
                            out_x[:, tcn], t2[:], b_rep[:], OP.add)
                    else:
                        nc.vector.tensor_scalar_mul(
                            out_x[:, tcn], t1[:], rstd[:, tcn:tcn + 1])

        def attention_b(b, qT, kT, v_sb, kv_chunks, oT, ca):
            """one batch of attention. qT/kT: [P, NF, T or SMB] f32r.
            v_sb: SA [P, NTB, NH, HD+1] / CA [SMB, NH, HD+1].
            oT: [P, NF, T] f32r out (unnormalized evac then in-place scale)."""
            sums = smallp.tile([NH, T], F32, tag="sums")

            def qk_stage(h):
                """scores + mask + exp for one head; returns exp tiles."""
                pr = (h % 2) * HD
                fc = h // 2
                ets = []
                for c2 in range(kv_chunks):
                    qlo = c2 * P if not ca else 0
                    qlen = T - qlo
                    if not ca:
                        klen = P
                        st = psp.tile([P, T], F32, tag="ps")
                        stv = st[:, :qlen]
                        ksl = kT[pr:pr + HD, fc, c2 * P:(c2 + 1) * P]
                    else:
                        klen = SM
                        st = psp.tile([SM, T], F32, tag="ps")
                        stv = st[:, :]
                        ksl = kT[pr:pr + HD, fc, b * SM:(b + 1) * SM]
                    nc.tensor.matmul(
                        stv, ksl, qT[pr:pr + HD, fc, qlo:T],
                        start=True, stop=True)
                    if not ca:
                        nc.vector.tensor_tensor(
                            st[:, :P], st[:, :P], msa[:], OP.add)
                    else:
                        nc.vector.tensor_tensor(stv, stv, mca[:], OP.add)
                    et = expp.tile([P, T], F32R, tag="expT")
                    nc.scalar.activation(et[:klen, :qlen], stv, AF.Exp)
                    ets.append((et, klen, qlo, qlen))
                return ets

            def av_stage(h, ets):
                pr = (h % 2) * HD
                fc = h // 2
                ops = psp.tile([HD + 1, T], F32, tag="ps")
                for c2, (et, klen, qlo, qlen) in enumerate(ets):
                    vsl = v_sb[:, c2, h] if not ca else v_sb[:, b, h]
                    nc.tensor.matmul(
                        ops[:, qlo:], vsl, et[:klen, :qlen],
                        start=(c2 == 0), stop=(c2 == kv_chunks - 1),
                        skip_group_check=True)
                stmp = smallp.tile([1, T], F32, tag="stmp")
                nc.scalar.copy(stmp[:], ops[HD:HD + 1, :])
                nc.sync.dma_start(sums[h:h + 1, :], stmp[:])
                # unnormalized evac (psum slot freed per head)
                if not ca:
                    nc.scalar.copy(oT[pr:pr + HD, fc, :], ops[:HD, :])
                else:
                    nc.vector.tensor_copy(oT[pr:pr + HD, fc, :], ops[:HD, :])

            # one-head software pipeline: PE runs head h+1's QK while head
            # h's mask/exp (DVE/ACT) complete, so PE never stalls on exp
            prev = None
            for h in range(NH):
                ets = qk_stage(h)
                if prev is not None:
                    av_stage(*prev)
                prev = (h, ets)
            av_stage(*prev)
            rs = smallp.tile([NH, T], F32R, tag="rs")
            with nc.allow_low_precision("f32r softmax reciprocal"):
                nc.vector.reciprocal(rs[:], sums[:])
            # replicate head recip rows across their 64 partitions via a K=8
            # selector matmul into PSUM; normalize reads PSUM directly
            for fc in range(NF):
                rsf = psp.tile([P, T], F32, tag="ps")
                nc.tensor.matmul(rsf[:], sel_sb[:, fc, :], rs[:],
                                 start=True, stop=True)
                nc.vector.tensor_tensor(
                    oT[:, fc, :], oT[:, fc, :], rsf[:], OP.mult)

        def out_proj_residual_b(b, oT, wo_sb, bo_row):
            for tl in range(NTB):
                tcn = b * NTB + tl
                pt = psp.tile([P, D], F32, tag="ps")
                for ic in range(NF):
                    nc.tensor.matmul(
                        pt[:], oT[:, ic, tl * P:(tl + 1) * P], wo_sb[:, ic, :],
                        start=(ic == 0),
                        stop=(ic == NF - 1 and bo_row is None))
                if bo_row is not None:
                    nc.tensor.matmul(pt[:], ones_r[:], bo_row[:],
                                     start=False, stop=True,
                                     skip_group_check=True)
                nc.vector.tensor_tensor(
                    r_sb[:, tcn], pt[:], x_sb[:, tcn], OP.add)

        # ---- embedding --------------------------------------------------
        for tcn in range(NT):
            pe_t = lnp.tile([P, D], F32, tag="t1")
            nc.sync.dma_start(
                pe_t[:], d_pe.ap()[(tcn % NTB) * P:((tcn % NTB) + 1) * P, :])
            pt = psp.tile([P, D], F32, tag="ps")
            nc.tensor.matmul(pt[:], tgtT[:, tcn * P:(tcn + 1) * P], embT[:],
                             start=True, stop=True)
            nc.vector.tensor_tensor(x_sb[:, tcn], pt[:], pe_t[:], OP.add)

        # ---- layers -----------------------------------------------------
        for l in range(n_layers):
            # ===== self attention =====
            xT = transpose_x(x_sb)
            wv_sb = load_w_full(d_wv, l, D, D, "wv")
            wo_sb = load_w_full(d_wo, l, D, D, "wo")
            bqc = bias_cols(d_bq, l, D, "bqc") if has_bias else None
            bkc = bias_cols(d_bk, l, D, "bkc") if has_bias else None
            bvr = load_b_row(d_bv, l, D, "bvr") if has_bias else None
            bor = load_b_row(d_bo, l, D, "bor") if has_bias else None
            for b in range(BL):
                qT = qkp.tile([P, NF, T], F32R, tag="qT")
                kT = qkp.tile([P, NF, T], F32R, tag="kT")
                proj_fmajor_b(xT, d_wq, l, b, qT, bqc, "wqc")
                proj_fmajor_b(xT, d_wk, l, b, kT, bkc, "wkc")
                v_sb = qkp.tile([P, NTB, NH, HD + 1], F32R, tag="v")
                nc.vector.tensor_copy(
                    v_sb[:, :, :, HD],
                    ones_pp[:, :NTB * NH].rearrange(
                        "p (a b) -> p a b", a=NTB, b=NH))
                for tl in range(NTB):
                    tcn = b * NTB + tl
                    pt = psp.tile([P, D], F32, tag="ps")
                    for ic in range(NF):
                        nc.tensor.matmul(
                            pt[:], xT[:, ic, tcn * P:(tcn + 1) * P],
                            wv_sb[:, ic, :],
                            start=(ic == 0),
                            stop=(ic == NF - 1 and bvr is None))
                    if bvr is not None:
                        nc.tensor.matmul(pt[:], ones_r[:], bvr[:],
                                         start=False, stop=True,
                                         skip_group_check=True)
                    nc.vector.tensor_copy(
                        v_sb[:, tl, :, 0:HD],
                        pt[:].rearrange("p (h d) -> p h d", h=NH))
                oT = otp.tile([P, NF, T], F32R, tag="oT")
                attention_b(b, qT, kT, v_sb, NTB, oT, ca=False)
                out_proj_residual_b(b, oT, wo_sb, bor)
            ln_block(3 * l + 0, x_sb)

            # ===== cross attention =====
            xT1 = transpose_x(x_sb)
            cv_sb = load_w_full(d_cv, l, D, D, "wv")
            co_sb = load_w_full(d_co, l, D, D, "wo")
            cbqc = bias_cols(d_cbq, l, D, "bqc") if has_bias else None
            cbkc = bias_cols(d_cbk, l, D, "bkc") if has_bias else None
            cbvr = load_b_row(d_cbv, l, D, "bvr") if has_bias else None
            cbor = load_b_row(d_cbo, l, D, "bor") if has_bias else None
            # memory k/v (both batches at once; tiny)
            kmT = qkp.tile([P, NF, SMB], F32R, tag="kmT")
            for oc in range(NF):
                ck_oc = stream_w_oc(d_ck, l, oc, "wkc")
                pt = psp.tile([P, SMB], F32, tag="ps")
                for ic in range(NF):
                    nc.tensor.matmul(
                        pt[:], ck_oc[:, ic, :], memT[:, ic, :],
                        start=(ic == 0), stop=(ic == NF - 1))
                if cbkc is not None:
                    nc.scalar.activation(
                        kmT[:, oc, :], pt[:], AF.Identity,
                        bias=cbkc[:, oc:oc + 1], scale=1.0)
                else:
                    nc.scalar.copy(kmT[:, oc, :], pt[:])
            vm = qkp.tile([SM, BL, NH, HD + 1], F32R, tag="vm")
            nc.vector.tensor_copy(
                vm[:, :, :, HD],
                ones_pp[:SM, :BL * NH].rearrange(
                    "p (a b) -> p a b", a=BL, b=NH))
            ptv = psp.tile([SMB, D], F32, tag="ps")
            for ic in range(NF):
                nc.tensor.matmul(
                    ptv[:], memT[:, ic, :], cv_sb[:, ic, :],
                    start=(ic == 0), stop=(ic == NF - 1 and cbvr is None))
            if cbvr is not None:
                nc.tensor.matmul(ptv[:], ones_r[:, :SMB], cbvr[:],
                                 start=False, stop=True, skip_group_check=True)
            for b in range(BL):
                nc.vector.tensor_copy(
                    vm[:, b, :, 0:HD],
                    ptv[b * SM:(b + 1) * SM].rearrange("p (h d) -> p h d", h=NH))
            for b in range(BL):
                qcT = qkp.tile([P, NF, T], F32R, tag="qT")
                proj_fmajor_b(xT1, d_cq, l, b, qcT, cbqc, "wqc")
                oTc = otp.tile([P, NF, T], F32R, tag="oT")
                attention_b(b, qcT, kmT, vm, 1, oTc, ca=True)
                out_proj_residual_b(b, oTc, co_sb, cbor)
            ln_block(3 * l + 1, x_sb)

            # ===== FFN (two fc-halves of 1024) =====
            xT2 = transpose_x(x_sb)
            b1c = bias_cols(d_b1, l, FF, "b1c") if has_bias else None
            b2r = load_b_row(d_b2, l, D, "bor") if has_bias else None
            for fh in range(2):
                hT = xtp.tile([P, NFH, TQ], F32R, tag="hT")
                for fc in range(NFH):
                    fg = fh * NFH + fc
                    w1c = wstr.tile([P, NF, P], F32R, tag="w1c")
                    nc.sync.dma_start(
                        w1c[:],
                        d_w1.ap()[l].rearrange("(c p) o -> p c o", p=P)
                        [:, :, fg * P:(fg + 1) * P])
                    for th in range(2):
                        pt = psp.tile([P, T], F32, tag="ps")
                        for ic in range(NF):
                            nc.tensor.matmul(
                                pt[:], w1c[:, ic, :],
                                xT2[:, ic, th * T:(th + 1) * T],
                                start=(ic == 0), stop=(ic == NF - 1))
                        nc.scalar.activation(
                            hT[:, fc, th * T:(th + 1) * T], pt[:], AF.Relu,
                            bias=(b1c[:, fg:fg + 1] if b1c is not None else 0.0),
                            scale=1.0)
                w2h = wpool.tile([P, NFH, D], F32R, tag="w2h")
                nc.sync.dma_start(
                    w2h[:],
                    d_w2.ap()[l, fh * FH:(fh + 1) * FH, :].rearrange(
                        "(c p) o -> p c o", p=P))
                for tcn in range(NT):
                    pt = psp.tile([P, D], F32, tag="ps")
                    extra_bias = (fh == 1 and has_bias)
                    for fc in range(NFH):
                        nc.tensor.matmul(
                            pt[:], hT[:, fc, tcn * P:(tcn + 1) * P],
                            w2h[:, fc, :],
                            start=(fc == 0),
                            stop=(fc == NFH - 1 and not extra_bias))
                    if fh == 1 and has_bias:
                        nc.tensor.matmul(pt[:], ones_r[:], b2r[:],
                                         start=False, stop=True,
                                         skip_group_check=True)
                    if fh == 0:
                        nc.vector.tensor_tensor(
                            r_sb[:, tcn], pt[:], x_sb[:, tcn], OP.add)
                    else:
                        nc.vector.tensor_tensor(
                            r_sb[:, tcn], pt[:], r_sb[:, tcn], OP.add)
            ln_block(3 * l + 2, x_sb)

        # ---- final LN + heads -------------------------------------------
        for tcn in range(NT):
            nc.vector.tensor_copy(r_sb[:, tcn], x_sb[:, tcn])
        ln_block(3 * L, x_sb)
        if debug_dump:
            nc.gpsimd.dma_start(
                d_dbg.ap().rearrange("(c p) o -> p c o", p=P), x_sb[:])
        xTf = transpose_x(x_sb)
        out_sb = pool.tile([P, NT, VOP], F32)
        for tcn in range(NT):
            pt = psp.tile([P, VOP], F32, tag="ps")
            for ic in range(NF):
                nc.tensor.matmul(
                    pt[:], xTf[:, ic, tcn * P:(tcn + 1) * P], hw_sb[:, ic, :],
                    start=(ic == 0), stop=(ic == NF - 1 and not has_bias))
            if has_bias:
                nc.tensor.matmul(pt[:], ones_r[:], hb_sb[:],
                                 start=False, stop=True, skip_group_check=True)
            nc.vector.tensor_copy(out_sb[:, tcn], pt[:])
        nc.sync.dma_start(
            d_out.ap().rearrange("(c p) o -> p c o", p=P),
            out_sb[:, :, :VO])

    nc.compile()
    return nc


# --------------------------------------------------------------------------
# host side
# --------------------------------------------------------------------------

def _pe_table():
    pos = np.arange(T)[:, None].astype(np.float32)
    div = np.exp(np.arange(0, D, 2).astype(np.float32) * (-np.log(10000.0) / D))
    pe = np.zeros((T, D), np.float32)
    pe[:, 0::2] = np.sin(pos * div)
    pe[:, 1::2] = np.cos(pos * div)
    return pe


def _masks():
    seg = np.arange(T) // 16
    allow = np.where(seg < 1, seg + 1, np.where(seg - 1 < 1, 1, seg))
    i = np.arange(P)
    msa = np.where(i[None, :] < i[:, None], NEG, 0.0).astype(np.float32)
    s = np.arange(SM)
    mca = np.where(s[:, None] >= allow[None, :], NEG, 0.0).astype(np.float32)
    return msa, mca


def _prep_inputs(inputs):
    f = lambda k: np.ascontiguousarray(np.asarray(inputs[k]), dtype=np.float32)
    tgt, memory = f("tgt"), f("memory")
    emb_w, emb_b = f("emb_w"), f("emb_b")
    sa_qkv_w, sa_qkv_b = f("sa_qkv_w"), f("sa_qkv_b")
    sa_out_w, sa_out_b = f("sa_out_w"), f("sa_out_b")
    ca_qkv_w, ca_qkv_b = f("ca_qkv_w"), f("ca_qkv_b")
    ca_out_w, ca_out_b = f("ca_out_w"), f("ca_out_b")
    ffn_w1, ffn_b1 = f("ffn_w1"), f("ffn_b1")
    ffn_w2, ffn_b2 = f("ffn_w2"), f("ffn_b2")
    ln_s_all = np.stack(
        [v for l in range(L) for v in
         (f("ln1_s")[l], f("ln2_s")[l], f("ln3_s")[l])] + [f("fn_s")]
    )[:, None, :]
    ln_b_all = np.stack(
        [v for l in range(L) for v in
         (f("ln1_b")[l], f("ln2_b")[l], f("ln3_b")[l])] + [f("fn_b")]
    )[:, None, :]
    hit_w, hit_b = f("hit_w"), f("hit_b")
    vel_w, vel_b = f("vel_w"), f("vel_b")
    off_w, off_b = f("off_w"), f("off_b")

    sc = 1.0 / np.sqrt(np.float32(HD))
    wq, wk, wv = sa_qkv_w[:, :D], sa_qkv_w[:, D:2 * D], sa_qkv_w[:, 2 * D:]
    bq, bk, bv = sa_qkv_b[:, :D], sa_qkv_b[:, D:2 * D], sa_qkv_b[:, 2 * D:]
    cwq, cwk, cwv = ca_qkv_w[:, :D], ca_qkv_w[:, D:2 * D], ca_qkv_w[:, 2 * D:]
    cbq, cbk, cbv = ca_qkv_b[:, :D], ca_qkv_b[:, D:2 * D], ca_qkv_b[:, 2 * D:]

    biases = [bq, bk, bv, sa_out_b, cbq, cbk, cbv, ca_out_b, ffn_b1, ffn_b2,
              hit_b, vel_b, off_b]
    has_bias = any(np.any(b) for b in biases)
    has_ln_scale = bool(np.any(ln_s_all != 1.0))
    has_ln_bias = bool(np.any(ln_b_all))
    flags = (has_bias, has_ln_scale, has_ln_bias)

    msa, mca = _masks()
    pe_eb = (_pe_table() + emb_b[None, :]).astype(np.float32)
    hsel = (np.arange(NH)[:, None] == (np.arange(D) // HD)[None, :]
            ).astype(np.float32)
    tr = lambda w: np.swapaxes(w, -1, -2)
    shared = dict(
        pe_eb=pe_eb, emb_wT=tr(emb_w), mask_sa=msa, mask_ca=mca,
        head_sel=hsel, identity=np.eye(P, dtype=np.float32),
        const_ones=np.ones((P, P), np.float32),
        wq=tr(wq) * sc, wk=tr(wk), wv=tr(wv), wo=tr(sa_out_w),
        cq=tr(cwq) * sc, ck=tr(cwk), cv=tr(cwv), co=tr(ca_out_w),
        w1=tr(ffn_w1), w2=tr(ffn_w2),
        hwT=np.pad(np.concatenate([hit_w, vel_w, off_w], 0).T,
                   ((0, 0), (0, VOP - VO))),
    )
    if has_bias:
        shared.update(
            bq=(bq * sc)[:, None, :], bk=bk[:, None, :], bv=bv[:, None, :],
            bo=sa_out_b[:, None, :],
            cbq=(cbq * sc)[:, None, :], cbk=cbk[:, None, :],
            cbv=cbv[:, None, :], cbo=ca_out_b[:, None, :],
            b1=ffn_b1[:, None, :], b2=ffn_b2[:, None, :],
            hb=np.pad(np.concatenate([hit_b, vel_b, off_b]), (0, VOP - VO))[None, :],
        )
    if has_ln_scale:
        shared["ln_s"] = ln_s_all
    if has_ln_bias:
        shared["ln_b"] = ln_b_all
    shared = {k: np.ascontiguousarray(v, dtype=np.float32)
              for k, v in shared.items()}

    in_maps = []
    for c in range(NCORES):
        bs = slice(c * BL, (c + 1) * BL)
        m = dict(shared)
        m["tgtT"] = np.ascontiguousarray(tgt[bs].reshape(TQ, VO).T)
        m["memT"] = np.ascontiguousarray(
            memory[bs, :SM, :].transpose(2, 0, 1).reshape(D, SMB))
        in_maps.append(m)
    return flags, in_maps


def kernel(**inputs):
    flags, in_maps = _prep_inputs(inputs)
    if flags not in _PROG_CACHE:
        _PROG_CACHE[flags] = _build_program(flags)
    nc = _PROG_CACHE[flags]
    res = run_bass_kernel_spmd(nc, in_maps, core_ids=list(range(NCORES)))
    outs = np.stack([r["out"].reshape(BL, T, VO) for r in res.results])
    full = outs.reshape(B, T, VO).astype(np.float32)
    return (np.ascontiguousarray(full[..., :V]),
            np.ascontiguousarray(full[..., V:2 * V]),
            np.ascontiguousarray(full[..., 2 * V:]))

